# revision 42
# baseline (speedup 1.0000x reference)
"""Trainium2 Bass kernel for the two-level Haar-DWT detail (L1) loss.

Strategy (pure data parallel over batch, 8 NeuronCores):
  - The host casts both inputs to bf16, deinterleaves columns (each
    512-wide image row becomes [even(256) | odd(256)]) and interleaves
    the two tensors row-wise into one [rows, o(512) | t(512)] array
    before upload (mixed-precision: the ~0.4% per-element rounding
    averages out to ~1e-4 on the final mean-abs loss, far inside
    tolerance) -- this halves the HBM stream to 12.6 MB/core (~35 us at
    the ~370 GB/s per-core DMA roofline), makes every DVE op fully
    packed, and needs only ONE 512 KiB DMA per pair-tile.
  - Each core gets 4 of the 32 batch images, viewed as a [6144, 1024]
    row matrix; 24 pair-tiles.  All loads are plain HWDGE DMAs issued
    from the Sync engine.  A few dummy matmuls during the framework
    preamble warm the PE up to its full p-state.
  - The loss is linear until the per-band |.|; the (x+1)/2 normalization
    of both inputs only scales d = output - target by 0.5 (host-folded).
  - d = o - t, and the level-1 column combines (cs/cd, reading the
    even/odd halves of d) are packed-bf16 DVE ops in 2x perf mode.
    (GpSimd/Pool shares SBUF ports with the DVE and cannot read PSUM,
    so it stays idle; the DVE cannot |.|-reduce PSUM in one op either,
    so the ScalarEngine keeps all the abs work.)
  - The TensorEngine does ALL remaining combines: w1q/w1 (the 0.1 LL1
    weight baked into w1q) fold the level-1 row combines, and the whole
    level-2 transform is two accumulating matmuls on strided rhs views
    (cs even / odd columns) with +-weight matrices [-W2 | +W2], yielding
    rows [HH2; HL2; LH2; 0] -- no level-2 DVE work at all.
  - The 2.5 PSUM banks of a pair form one clean [128, 1280] rectangle
    (zero-padded weights keep rows 96:128 of bank 2 exact zeros): a
    single ScalarEngine Abs-activation with accum_out per pair produces
    the per-partition abs-sums (level-1 |4*band1| and level-2 |8*band2|
    share the same 1/(4*n1) loss divisor, so one accumulator suffices).
  - Each core emits [128, 4]; host combines in float64.
"""

import numpy as np

B, C, H, W = 32, 3, 512, 512
N_CORES = 8
B_PER_CORE = B // N_CORES
ROWS = B_PER_CORE * C * H  # 6144
COLS = W  # 512
NT = ROWS // 128  # 48 tiles per core
NP = NT // 2  # 24 tile-pairs

_CACHE = {}


def _make_weights():
    import ml_dtypes
    q = ml_dtypes.bfloat16(0.1)  # LL1 loss weight, baked into W1q
    # w1q[k, m]: row pair-combine for the S (col-sum) path.
    # m<64: +q at rows 2m, 2m+1 (pair sum -> LL1, pre-weighted);
    # m=64+mm: -1/+1 (pair diff -> LH1).
    w1q = np.zeros((128, 128), ml_dtypes.bfloat16)
    w1 = np.zeros((128, 128), ml_dtypes.bfloat16)
    for m in range(64):
        w1q[2 * m, m] = q
        w1q[2 * m + 1, m] = q
        w1q[2 * m, 64 + m] = -1.0
        w1q[2 * m + 1, 64 + m] = 1.0
        # plain +-1 for the D (col-diff) path: HL1 | HH1
        w1[2 * m, m] = 1.0
        w1[2 * m + 1, m] = 1.0
        w1[2 * m, 64 + m] = -1.0
        w1[2 * m + 1, 64 + m] = 1.0
    # Level 2 in two accumulating matmuls: psum2 = w2neg @ cs_even +
    # w2pos @ cs_odd.  Rows: [HH2 (diff of cd2); HL2 (sum of cd2);
    # LH2 (diff of cs2); 0], with cd2 = cs_o - cs_e, cs2 = cs_e + cs_o.
    # Zero-padded to 128 outputs so psum rows 96:128 are exact zeros.
    w2neg = np.zeros((128, 128), ml_dtypes.bfloat16)
    w2pos = np.zeros((128, 128), ml_dtypes.bfloat16)
    for m in range(32):
        for r in range(4):
            sd = -1.0 if r < 2 else 1.0  # 4-row diff pattern
            row = 4 * m + r
            w2neg[row, m] = -sd          # HH2: w24d . (cs_o - cs_e)
            w2pos[row, m] = sd
            w2neg[row, 32 + m] = -1.0    # HL2: w24s . (cs_o - cs_e)
            w2pos[row, 32 + m] = 1.0
            w2neg[row, 64 + m] = sd      # LH2: w24d . (cs_e + cs_o)
            w2pos[row, 64 + m] = sd
    return w1q, w1, w2neg, w2pos


def _build_bass():
    from contextlib import ExitStack

    import concourse.bacc as bacc
    import concourse.bass as bass
    import concourse.mybir as mybir
    import concourse.tile as tile

    F32 = mybir.dt.float32
    BF16 = mybir.dt.bfloat16
    X = mybir.AxisListType.X
    ADD = mybir.AluOpType.add
    ABS_MAX = mybir.AluOpType.abs_max
    ABS = mybir.ActivationFunctionType.Abs

    nc = bacc.Bacc("TRN2", target_bir_lowering=False, debug=False,
                   num_devices=N_CORES)
    ot_d = nc.dram_tensor("ot", [ROWS, 2 * COLS], BF16,
                          kind="ExternalInput").ap()
    w1q_d = nc.dram_tensor("w1q", [128, 128], BF16, kind="ExternalInput").ap()
    w1_d = nc.dram_tensor("w1", [128, 128], BF16, kind="ExternalInput").ap()
    w2n_d = nc.dram_tensor("w2n", [128, 128], BF16, kind="ExternalInput").ap()
    w2p_d = nc.dram_tensor("w2p", [128, 128], BF16, kind="ExternalInput").ap()
    res_d = nc.dram_tensor("res", [128, 4], F32, kind="ExternalOutput").ap()

    # DRAM view for 512 KiB pair loads: [part, block, col] (the SBUF side
    # is one flat 4 KiB run per partition).
    W2 = 2 * COLS
    def dram_view(ap, pr):
        return bass.AP(tensor=ap.tensor, offset=pr * 2 * 128 * W2,
                       ap=[[W2, 128], [128 * W2, 2], [1, W2]])

    with tile.TileContext(nc) as tc, ExitStack() as ctx:
        consts = ctx.enter_context(tc.tile_pool(name="consts", bufs=1))
        loads = ctx.enter_context(tc.tile_pool(name="loads", bufs=7))
        dpool = ctx.enter_context(tc.tile_pool(name="dpool", bufs=4))
        bands = ctx.enter_context(tc.tile_pool(name="bands", bufs=4))
        absout = ctx.enter_context(tc.tile_pool(name="absout", bufs=1))
        psP = ctx.enter_context(tc.tile_pool(name="psP", bufs=2, space="PSUM"))
        accp = ctx.enter_context(tc.tile_pool(name="accp", bufs=1))

        w1q_t = consts.tile([128, 128], BF16)
        w1_t = consts.tile([128, 128], BF16)
        w2n_t = consts.tile([128, 128], BF16)
        w2p_t = consts.tile([128, 128], BF16)

        acc1 = accp.tile([128, NP], F32)
        mm = nc.tensor.matmul

        # Warm the PE to full p-state during the framework preamble:
        # a few dummy matmuls on a zeroed tile into a scratch psum.
        warm = consts.tile([128, 512], BF16)
        nc.gpsimd.memset(warm[:], 0.0)
        pswarm = psP.tile([128, 512], F32, tag="warm", bufs=1)
        for _ in range(6):
            mm(pswarm[:], lhsT=warm[:, 0:128], rhs=warm[:],
               start=True, stop=True)

        for pr in range(NP):
            ot = loads.tile([128, 4 * COLS], BF16, tag="ot")
            # The first loads also use the Scalar HWDGE queue so the DMA
            # ring fills at 2x the single-queue issue rate at startup.
            eng = nc.scalar if pr in (1, 3) else nc.sync
            eng.dma_start(ot[:].rearrange("p (b c) -> p b c", b=2),
                          dram_view(ot_d, pr))
            if pr == 0:
                # Weights right after the first data load (but before
                # any consumer in program order): the stream starts
                # immediately and the weights still arrive in time.
                nc.sync.dma_start(w1q_t[:], w1q_d)
                nc.sync.dma_start(w1_t[:], w1_d)
                nc.sync.dma_start(w2n_t[:], w2n_d)
                nc.sync.dma_start(w2p_t[:], w2p_d)

            # d = o - t: one packed-bf16 DVE op (2x perf mode) on the
            # row-interleaved [block, o|t, col] layout.
            d = dpool.tile([128, 2 * COLS], BF16, tag="d")
            d3 = d[:].rearrange("p (b c) -> p b c", b=2)
            ot4 = ot[:].rearrange("p (b s c) -> p b s c", b=2, s=2)
            nc.vector.tensor_sub(d3, ot4[:, :, 0, :], ot4[:, :, 1, :])

            # level-1 column combines; the host deinterleave makes these
            # packed too (out [128, block, 256] vs the d halves),
            # keeping the DVE in 2x mode.  cs and cd share one tile.
            csd = bands.tile([128, 2 * COLS], BF16, tag="csd")
            cs = csd[:, 0:COLS]
            cd = csd[:, COLS:2 * COLS]
            cs3 = cs.rearrange("p (b c) -> p b c", b=2)
            cd3 = cd.rearrange("p (b c) -> p b c", b=2)
            d3 = d[:].rearrange("p (b c) -> p b c", b=2)
            de = d3[:, :, 0:COLS // 2]
            do = d3[:, :, COLS // 2:COLS]
            nc.vector.tensor_add(cs3, de, do)
            nc.vector.tensor_sub(cd3, do, de)

            # Row combines on the PE.  Bank 0 = S (LL1|LH1), bank 1 = D
            # (HL1|HH1), bank 2 first half = level-2 rows [HH2; HL2;
            # LH2; 0] built by two accumulating matmuls on cs even/odd.
            psumP = psP.tile([128, 1536], F32)
            mm(psumP[:, 0:512], lhsT=w1q_t[:], rhs=cs,
               start=True, stop=True)
            mm(psumP[:, 512:1024], lhsT=w1_t[:], rhs=cd,
               start=True, stop=True)
            mm(psumP[:, 1024:1280], lhsT=w2n_t[:], rhs=cs[:, 0:COLS:2],
               start=True, stop=False)
            mm(psumP[:, 1024:1280], lhsT=w2p_t[:], rhs=cs[:, 1:COLS:2],
               start=False, stop=True)

            # One fused |.| + per-partition sum over the 2.5 psum banks.
            ab = absout.tile([128, 1280], BF16, tag="ab")
            nc.scalar.activation(ab[:], psumP[:, 0:1280], ABS,
                                 accum_out=acc1[:, pr:pr + 1])

        res_t = accp.tile([128, 4], F32)
        nc.vector.memset(res_t[:], 0.0)
        nc.vector.tensor_reduce(res_t[:, 0:1], acc1[:], axis=X, op=ADD)
        nc.sync.dma_start(res_d, res_t[:])

    nc.compile()
    return nc


def _get_bass():
    if "nc" not in _CACHE:
        _CACHE["nc"] = _build_bass()
    return _CACHE["nc"]


def _numpy_reference(output, target):
    """Full-precision fallback (only for the never-hit mixed-normalize case)."""
    o = output.astype(np.float64)
    t = target.astype(np.float64)
    if o.min() < 0:
        o = (o + 1.0) * 0.5
    if t.min() < 0:
        t = (t + 1.0) * 0.5

    def dwt(x):
        a = x[:, :, 0::2, 0::2]
        b = x[:, :, 0::2, 1::2]
        c = x[:, :, 1::2, 0::2]
        d = x[:, :, 1::2, 1::2]
        return (0.5 * (a + b + c + d), 0.5 * (-a - b + c + d),
                0.5 * (-a + b - c + d), 0.5 * (a - b - c + d))

    ll_o, lh_o, hl_o, hh_o = dwt(o)
    ll_t, lh_t, hl_t, hh_t = dwt(t)
    tot = (np.abs(lh_o - lh_t).mean() + np.abs(hl_o - hl_t).mean()
           + np.abs(hh_o - hh_t).mean() + 0.1 * np.abs(ll_o - ll_t).mean())
    _, lh2_o, hl2_o, hh2_o = dwt(ll_o)
    _, lh2_t, hl2_t, hh2_t = dwt(ll_t)
    tot += 0.5 * (np.abs(lh2_o - lh2_t).mean() + np.abs(hl2_o - hl2_t).mean()
                  + np.abs(hh2_o - hh2_t).mean())
    return np.float32(tot)


def _run_device(o, t, trace=False):
    """Shard [32,3,512,512] f32 arrays over 8 cores and run the Bass NEFF."""
    import ml_dtypes
    from concourse.bass_utils import run_bass_kernel_spmd

    nc = _get_bass()
    w1q, w1, w2neg, w2pos = _make_weights()

    # bf16 cast + per-row column deinterleave ([even(256) | odd(256)])
    # + row-interleave of the two tensors: row r = [o_deint | t_deint].
    N = B * C * H
    otb = np.empty((N, 2 * COLS), ml_dtypes.bfloat16)
    for i, x in enumerate((o, t)):
        xb = x.reshape(N, COLS).astype(ml_dtypes.bfloat16)
        base = i * COLS
        otb[:, base:base + COLS // 2] = xb[:, 0::2]
        otb[:, base + COLS // 2:base + COLS] = xb[:, 1::2]
    otb = otb.reshape(B, C * H, 2 * COLS)
    in_maps = []
    for c in range(N_CORES):
        sl = slice(c * B_PER_CORE, (c + 1) * B_PER_CORE)
        in_maps.append({
            "ot": otb[sl].reshape(ROWS, 2 * COLS),
            "w1q": w1q, "w1": w1, "w2n": w2neg, "w2p": w2pos,
        })
    res = run_bass_kernel_spmd(nc, in_maps, core_ids=list(range(N_CORES)),
                               trace=trace)
    _CACHE["last_result"] = res
    return res


def combine(results, both_norm=True):
    """Combine per-core [128, 4] abs-sum tensors into the scalar loss."""
    m = 0.0
    for r in results:
        m += r.astype(np.float64)[:, 0].sum()
    n1 = float(B * C * (H // 2) * (W // 2))
    scale = 4.0 * n1 if both_norm else 2.0 * n1
    return np.float32(m / scale)


def kernel(output, target):
    o = np.ascontiguousarray(np.asarray(output, dtype=np.float32))
    t = np.ascontiguousarray(np.asarray(target, dtype=np.float32))
    o_norm = bool(o.min() < 0.0)
    t_norm = bool(t.min() < 0.0)
    if o_norm != t_norm:
        # Normalization applied to only one input: the difference is no
        # longer a pure scale of o - t.  Practically unreachable for the
        # randn inputs this problem uses.
        return _numpy_reference(o, t)

    results = [r["res"] for r in _run_device(o, t).results]
    return combine(results, both_norm=o_norm)


# revision 43
# speedup vs baseline: 1.0561x; 1.0561x over previous
"""Trainium2 Bass kernel for the two-level Haar-DWT detail (L1) loss.

Strategy (pure data parallel over batch, 8 NeuronCores):
  - The host casts both inputs to bf16, deinterleaves columns (each
    512-wide image row becomes [even(256) | odd(256)]) and interleaves
    the two tensors row-wise into one [rows, o(512) | t(512)] array
    before upload (mixed-precision: the ~0.4% per-element rounding
    averages out to ~1e-4 on the final mean-abs loss, far inside
    tolerance) -- this halves the HBM stream to 12.6 MB/core (~35 us at
    the ~370 GB/s per-core DMA roofline), makes every DVE op fully
    packed, and needs only ONE 512 KiB DMA per pair-tile.
  - Each core gets 4 of the 32 batch images, viewed as a [6144, 1024]
    row matrix; 24 pair-tiles.  All loads are plain HWDGE DMAs issued
    from the Sync engine.  A few dummy matmuls during the framework
    preamble warm the PE up to its full p-state.
  - The loss is linear until the per-band |.|; the (x+1)/2 normalization
    of both inputs only scales d = output - target by 0.5 (host-folded).
  - d = o - t, and the level-1 column combines (cs/cd, reading the
    even/odd halves of d) are packed-bf16 DVE ops in 2x perf mode.
    (GpSimd/Pool shares SBUF ports with the DVE and cannot read PSUM,
    so it stays idle; the DVE cannot |.|-reduce PSUM in one op either,
    so the ScalarEngine keeps all the abs work.)
  - The TensorEngine does ALL remaining combines: w1q/w1 (the 0.1 LL1
    weight baked into w1q) fold the level-1 row combines, and the whole
    level-2 transform is two accumulating matmuls on strided rhs views
    (cs even / odd columns) with +-weight matrices [-W2 | +W2], yielding
    rows [HH2; HL2; LH2; 0] -- no level-2 DVE work at all.
  - The 2.5 PSUM banks of a pair form one clean [128, 1280] rectangle
    (zero-padded weights keep rows 96:128 of bank 2 exact zeros): a
    single ScalarEngine Abs-activation with accum_out per pair produces
    the per-partition abs-sums (level-1 |4*band1| and level-2 |8*band2|
    share the same 1/(4*n1) loss divisor, so one accumulator suffices).
  - Each core emits [128, 4]; host combines in float64.
"""

import numpy as np

B, C, H, W = 32, 3, 512, 512
N_CORES = 8
B_PER_CORE = B // N_CORES
ROWS = B_PER_CORE * C * H  # 6144
COLS = W  # 512
NT = ROWS // 128  # 48 tiles per core
NP = NT // 2  # 24 tile-pairs

_CACHE = {}


def _make_weights():
    import ml_dtypes
    q = ml_dtypes.bfloat16(0.1)  # LL1 loss weight, baked into W1q
    # w1q[k, m]: row pair-combine for the S (col-sum) path.
    # m<64: +q at rows 2m, 2m+1 (pair sum -> LL1, pre-weighted);
    # m=64+mm: -1/+1 (pair diff -> LH1).
    w1q = np.zeros((128, 128), ml_dtypes.bfloat16)
    w1 = np.zeros((128, 128), ml_dtypes.bfloat16)
    for m in range(64):
        w1q[2 * m, m] = q
        w1q[2 * m + 1, m] = q
        w1q[2 * m, 64 + m] = -1.0
        w1q[2 * m + 1, 64 + m] = 1.0
        # plain +-1 for the D (col-diff) path: HL1 | HH1
        w1[2 * m, m] = 1.0
        w1[2 * m + 1, m] = 1.0
        w1[2 * m, 64 + m] = -1.0
        w1[2 * m + 1, 64 + m] = 1.0
    # Level 2 in two accumulating matmuls: psum2 = w2neg @ cs_even +
    # w2pos @ cs_odd.  Rows: [HH2 (diff of cd2); HL2 (sum of cd2);
    # LH2 (diff of cs2); 0], with cd2 = cs_o - cs_e, cs2 = cs_e + cs_o.
    # Zero-padded to 128 outputs so psum rows 96:128 are exact zeros.
    w2neg = np.zeros((128, 128), ml_dtypes.bfloat16)
    w2pos = np.zeros((128, 128), ml_dtypes.bfloat16)
    for m in range(32):
        for r in range(4):
            sd = -1.0 if r < 2 else 1.0  # 4-row diff pattern
            row = 4 * m + r
            w2neg[row, m] = -sd          # HH2: w24d . (cs_o - cs_e)
            w2pos[row, m] = sd
            w2neg[row, 32 + m] = -1.0    # HL2: w24s . (cs_o - cs_e)
            w2pos[row, 32 + m] = 1.0
            w2neg[row, 64 + m] = sd      # LH2: w24d . (cs_e + cs_o)
            w2pos[row, 64 + m] = sd
    return w1q, w1, w2neg, w2pos


def _build_bass():
    from contextlib import ExitStack

    import concourse.bacc as bacc
    import concourse.bass as bass
    import concourse.mybir as mybir
    import concourse.tile as tile

    F32 = mybir.dt.float32
    BF16 = mybir.dt.bfloat16
    X = mybir.AxisListType.X
    ADD = mybir.AluOpType.add
    ABS_MAX = mybir.AluOpType.abs_max
    ABS = mybir.ActivationFunctionType.Abs

    nc = bacc.Bacc("TRN2", target_bir_lowering=False, debug=False,
                   num_devices=N_CORES)
    ot_d = nc.dram_tensor("ot", [ROWS, 2 * COLS], BF16,
                          kind="ExternalInput").ap()
    w1q_d = nc.dram_tensor("w1q", [128, 128], BF16, kind="ExternalInput").ap()
    w1_d = nc.dram_tensor("w1", [128, 128], BF16, kind="ExternalInput").ap()
    w2n_d = nc.dram_tensor("w2n", [128, 128], BF16, kind="ExternalInput").ap()
    w2p_d = nc.dram_tensor("w2p", [128, 128], BF16, kind="ExternalInput").ap()
    res_d = nc.dram_tensor("res", [128, 4], F32, kind="ExternalOutput").ap()

    # DRAM view for 512 KiB pair loads: [part, block, col] (the SBUF side
    # is one flat 4 KiB run per partition).
    W2 = 2 * COLS
    def dram_view(ap, pr):
        return bass.AP(tensor=ap.tensor, offset=pr * 2 * 128 * W2,
                       ap=[[W2, 128], [128 * W2, 2], [1, W2]])

    with tile.TileContext(nc) as tc, ExitStack() as ctx:
        consts = ctx.enter_context(tc.tile_pool(name="consts", bufs=1))
        loads = ctx.enter_context(tc.tile_pool(name="loads", bufs=7))
        dpool = ctx.enter_context(tc.tile_pool(name="dpool", bufs=4))
        bands = ctx.enter_context(tc.tile_pool(name="bands", bufs=4))
        absout = ctx.enter_context(tc.tile_pool(name="absout", bufs=3))
        psP = ctx.enter_context(tc.tile_pool(name="psP", bufs=2, space="PSUM"))
        accp = ctx.enter_context(tc.tile_pool(name="accp", bufs=1))

        w1q_t = consts.tile([128, 128], BF16)
        w1_t = consts.tile([128, 128], BF16)
        w2n_t = consts.tile([128, 128], BF16)
        w2p_t = consts.tile([128, 128], BF16)

        acc1 = accp.tile([128, NP], F32)
        mm = nc.tensor.matmul

        # Warm the PE to full p-state during the framework preamble:
        # a few dummy matmuls on a zeroed tile into a scratch psum.
        warm = consts.tile([128, 512], BF16)
        nc.gpsimd.memset(warm[:], 0.0)
        pswarm = psP.tile([128, 512], F32, tag="warm", bufs=1)
        for _ in range(6):
            mm(pswarm[:], lhsT=warm[:, 0:128], rhs=warm[:],
               start=True, stop=True)

        for pr in range(NP):
            ot = loads.tile([128, 4 * COLS], BF16, tag="ot")
            # The first loads also use the Scalar HWDGE queue so the DMA
            # ring fills at 2x the single-queue issue rate at startup.
            eng = nc.scalar if pr in (1, 3) else nc.sync
            eng.dma_start(ot[:].rearrange("p (b c) -> p b c", b=2),
                          dram_view(ot_d, pr))
            if pr == 0:
                # Weights right after the first data load (but before
                # any consumer in program order): the stream starts
                # immediately and the weights still arrive in time.
                nc.sync.dma_start(w1q_t[:], w1q_d)
                nc.sync.dma_start(w1_t[:], w1_d)
                nc.sync.dma_start(w2n_t[:], w2n_d)
                nc.sync.dma_start(w2p_t[:], w2p_d)

            # d = o - t: one packed-bf16 DVE op (2x perf mode) on the
            # row-interleaved [block, o|t, col] layout.
            d = dpool.tile([128, 2 * COLS], BF16, tag="d")
            d3 = d[:].rearrange("p (b c) -> p b c", b=2)
            ot4 = ot[:].rearrange("p (b s c) -> p b s c", b=2, s=2)
            nc.vector.tensor_sub(d3, ot4[:, :, 0, :], ot4[:, :, 1, :])

            # level-1 column combines; the host deinterleave makes these
            # packed too (out [128, block, 256] vs the d halves),
            # keeping the DVE in 2x mode.  cs and cd share one tile.
            csd = bands.tile([128, 2 * COLS], BF16, tag="csd")
            cs = csd[:, 0:COLS]
            cd = csd[:, COLS:2 * COLS]
            cs3 = cs.rearrange("p (b c) -> p b c", b=2)
            cd3 = cd.rearrange("p (b c) -> p b c", b=2)
            d3 = d[:].rearrange("p (b c) -> p b c", b=2)
            de = d3[:, :, 0:COLS // 2]
            do = d3[:, :, COLS // 2:COLS]
            nc.vector.tensor_add(cs3, de, do)
            nc.vector.tensor_sub(cd3, do, de)

            # Row combines on the PE.  Bank 0 = S (LL1|LH1), bank 1 = D
            # (HL1|HH1), bank 2 first half = level-2 rows [HH2; HL2;
            # LH2; 0] built by two accumulating matmuls on cs even/odd.
            psumP = psP.tile([128, 1536], F32)
            mm(psumP[:, 0:512], lhsT=w1q_t[:], rhs=cs,
               start=True, stop=True)
            mm(psumP[:, 512:1024], lhsT=w1_t[:], rhs=cd,
               start=True, stop=True)
            mm(psumP[:, 1024:1280], lhsT=w2n_t[:], rhs=cs[:, 0:COLS:2],
               start=True, stop=False)
            mm(psumP[:, 1024:1280], lhsT=w2p_t[:], rhs=cs[:, 1:COLS:2],
               start=False, stop=True)

            # One fused |.| + per-partition sum over the 2.5 psum banks.
            ab = absout.tile([128, 1280], BF16, tag="ab")
            nc.scalar.activation(ab[:], psumP[:, 0:1280], ABS,
                                 accum_out=acc1[:, pr:pr + 1])

        res_t = accp.tile([128, 4], F32)
        nc.vector.memset(res_t[:], 0.0)
        nc.vector.tensor_reduce(res_t[:, 0:1], acc1[:], axis=X, op=ADD)
        nc.sync.dma_start(res_d, res_t[:])

    nc.compile()
    return nc


def _get_bass():
    if "nc" not in _CACHE:
        _CACHE["nc"] = _build_bass()
    return _CACHE["nc"]


def _numpy_reference(output, target):
    """Full-precision fallback (only for the never-hit mixed-normalize case)."""
    o = output.astype(np.float64)
    t = target.astype(np.float64)
    if o.min() < 0:
        o = (o + 1.0) * 0.5
    if t.min() < 0:
        t = (t + 1.0) * 0.5

    def dwt(x):
        a = x[:, :, 0::2, 0::2]
        b = x[:, :, 0::2, 1::2]
        c = x[:, :, 1::2, 0::2]
        d = x[:, :, 1::2, 1::2]
        return (0.5 * (a + b + c + d), 0.5 * (-a - b + c + d),
                0.5 * (-a + b - c + d), 0.5 * (a - b - c + d))

    ll_o, lh_o, hl_o, hh_o = dwt(o)
    ll_t, lh_t, hl_t, hh_t = dwt(t)
    tot = (np.abs(lh_o - lh_t).mean() + np.abs(hl_o - hl_t).mean()
           + np.abs(hh_o - hh_t).mean() + 0.1 * np.abs(ll_o - ll_t).mean())
    _, lh2_o, hl2_o, hh2_o = dwt(ll_o)
    _, lh2_t, hl2_t, hh2_t = dwt(ll_t)
    tot += 0.5 * (np.abs(lh2_o - lh2_t).mean() + np.abs(hl2_o - hl2_t).mean()
                  + np.abs(hh2_o - hh2_t).mean())
    return np.float32(tot)


def _run_device(o, t, trace=False):
    """Shard [32,3,512,512] f32 arrays over 8 cores and run the Bass NEFF."""
    import ml_dtypes
    from concourse.bass_utils import run_bass_kernel_spmd

    nc = _get_bass()
    w1q, w1, w2neg, w2pos = _make_weights()

    # bf16 cast + per-row column deinterleave ([even(256) | odd(256)])
    # + row-interleave of the two tensors: row r = [o_deint | t_deint].
    N = B * C * H
    otb = np.empty((N, 2 * COLS), ml_dtypes.bfloat16)
    for i, x in enumerate((o, t)):
        xb = x.reshape(N, COLS).astype(ml_dtypes.bfloat16)
        base = i * COLS
        otb[:, base:base + COLS // 2] = xb[:, 0::2]
        otb[:, base + COLS // 2:base + COLS] = xb[:, 1::2]
    otb = otb.reshape(B, C * H, 2 * COLS)
    in_maps = []
    for c in range(N_CORES):
        sl = slice(c * B_PER_CORE, (c + 1) * B_PER_CORE)
        in_maps.append({
            "ot": otb[sl].reshape(ROWS, 2 * COLS),
            "w1q": w1q, "w1": w1, "w2n": w2neg, "w2p": w2pos,
        })
    res = run_bass_kernel_spmd(nc, in_maps, core_ids=list(range(N_CORES)),
                               trace=trace)
    _CACHE["last_result"] = res
    return res


def combine(results, both_norm=True):
    """Combine per-core [128, 4] abs-sum tensors into the scalar loss."""
    m = 0.0
    for r in results:
        m += r.astype(np.float64)[:, 0].sum()
    n1 = float(B * C * (H // 2) * (W // 2))
    scale = 4.0 * n1 if both_norm else 2.0 * n1
    return np.float32(m / scale)


def kernel(output, target):
    o = np.ascontiguousarray(np.asarray(output, dtype=np.float32))
    t = np.ascontiguousarray(np.asarray(target, dtype=np.float32))
    o_norm = bool(o.min() < 0.0)
    t_norm = bool(t.min() < 0.0)
    if o_norm != t_norm:
        # Normalization applied to only one input: the difference is no
        # longer a pure scale of o - t.  Practically unreachable for the
        # randn inputs this problem uses.
        return _numpy_reference(o, t)

    results = [r["res"] for r in _run_device(o, t).results]
    return combine(results, both_norm=o_norm)
